# revision 3
# baseline (speedup 1.0000x reference)
"""LoRA basis-bank kernel for 8 TRN2 NeuronCores.

Math (per batch b):
    A_mixed  = sum_k alpha[b,k] * A_bank[k]        # [R, DIN]
    B_mixedT = sum_k alpha[b,k] * B_bank[k].T      # [R, DOUT]
    z        = h[b] @ A_mixed.T                    # [S, R]
    delta[b] = z @ B_mixedT                        # [S, DOUT]

Sharding: data-parallel over batch, 1 batch per core; banks replicated.

Device dataflow per core:
  - mix banks with a single matmul against a [K*R, R] block-diagonal
    alpha placement matrix (built on host -- data placement only)
  - transpose h in 128x128 blocks on the tensor engine (fp32, exact)
  - mm1: zT[r, s] accumulated over DIN chunks, float32r (fast fp32)
  - mm2: delta[s, o] = zT.T @ B_mixedT, float32r
"""

import numpy as np

import concourse.bacc as bacc
import concourse.bass as bass
import concourse.mybir as mybir
import concourse.tile as tile
from concourse.bass_utils import run_bass_kernel_spmd
from concourse.masks import make_identity

B, S, K, R, DIN, DOUT = 8, 2048, 16, 16, 2048, 2048
KR = K * R  # 256
F32 = mybir.dt.float32
F32R = mybir.dt.float32r

_cache = {}


def _build_nc():
    nc = bacc.Bacc("TRN2", target_bir_lowering=False)

    h_d = nc.dram_tensor("hb", [S, DIN], F32, kind="ExternalInput")
    mix_d = nc.dram_tensor("mix", [KR, R], F32, kind="ExternalInput")
    a_d = nc.dram_tensor("a_flat", [KR, DIN], F32, kind="ExternalInput")
    bt_d = nc.dram_tensor("bt_flat", [KR, DOUT], F32, kind="ExternalInput")
    out_d = nc.dram_tensor("delta", [S, DOUT], F32, kind="ExternalOutput")

    with tile.TileContext(nc) as tc:
        with (
            tc.tile_pool(name="const", bufs=1) as constp,
            tc.tile_pool(name="banks", bufs=1) as bankp,
            tc.tile_pool(name="hin", bufs=6) as hp,
            tc.tile_pool(name="hT", bufs=2) as hTp,
            tc.tile_pool(name="zz", bufs=2) as zp,
            tc.tile_pool(name="dout", bufs=3) as dp,
            tc.tile_pool(name="pst", bufs=2, space="PSUM") as pstp,
            tc.tile_pool(name="psz", bufs=2, space="PSUM") as pszp,
            tc.tile_pool(name="psd", bufs=3, space="PSUM") as psdp,
        ):
            ident = constp.tile([128, 128], F32, tag="ident")
            make_identity(nc, ident[:])

            # ---- prologue: load banks + alpha placement matrix ----
            m_sb = []
            for half in range(2):
                m_t = constp.tile([128, R], F32, tag=f"m{half}")
                nc.sync.dma_start(m_t[:], mix_d[half * 128:(half + 1) * 128, :])
                m_sb.append(m_t)
            a_sb, b_sb = [], []
            for half in range(2):
                a_t = bankp.tile([128, DIN], F32, tag=f"a{half}")
                nc.sync.dma_start(a_t[:], a_d[half * 128:(half + 1) * 128, :])
                a_sb.append(a_t)
                b_t = bankp.tile([128, DOUT], F32, tag=f"b{half}")
                nc.sync.dma_start(b_t[:], bt_d[half * 128:(half + 1) * 128, :])
                b_sb.append(b_t)

            # ---- mix banks: out[r, :] = sum_{k,r'} Mx[(k,r'), r] * bank[(k,r'), :]
            amixed = constp.tile([R, DIN], F32, tag="amixed")
            bmixT = constp.tile([R, DOUT], F32R, tag="bmixT")
            for dst, src in ((amixed, a_sb), (bmixT, b_sb)):
                for c4 in range(DIN // 512):
                    sl = slice(c4 * 512, (c4 + 1) * 512)
                    pmix = pstp.tile([R, 512], F32, tag="pt")
                    nc.tensor.matmul(pmix[:], m_sb[0][:], src[0][:, sl],
                                     start=True, stop=False)
                    nc.tensor.matmul(pmix[:], m_sb[1][:], src[1][:, sl],
                                     start=False, stop=True)
                    nc.vector.tensor_copy(dst[:, sl], pmix[:])

            # ---- A_mixed.T chunks: [128, R] x 16 (lhsT for mm1) ----
            amixT = []
            for c in range(DIN // 128):
                pat = pszp.tile([128, R], F32, tag="zt")
                nc.tensor.matmul(pat[:], amixed[:, c * 128:(c + 1) * 128],
                                 ident[:R, :R], is_transpose=True)
                t_sb = constp.tile([128, R], F32R, tag=f"amixT{c}")
                nc.vector.tensor_copy(t_sb[:], pat[:])
                amixT.append(t_sb)

            # ---- main loop over s-chunks of 512 rows ----
            for sc in range(S // 512):
                hts = []
                for t in range(4):
                    row0 = (sc * 4 + t) * 128
                    ht = hp.tile([128, DIN], F32, tag="h")
                    nc.sync.dma_start(ht[:], h_d[row0:row0 + 128, :])
                    hts.append(ht)

                # transpose h: hT[c][i, s] for each 128-wide DIN chunk c
                hTs = []
                for c in range(DIN // 128):
                    pt = pstp.tile([128, 512], F32, tag="pt")
                    for t in range(4):
                        nc.tensor.matmul(
                            pt[:, t * 128:(t + 1) * 128],
                            hts[t][:, c * 128:(c + 1) * 128],
                            ident[:], is_transpose=True)
                    hT = hTp.tile([128, 512], F32R, tag=f"hT{c}")
                    nc.vector.tensor_copy(hT[:], pt[:])
                    hTs.append(hT)

                # mm1: zT [R, 512] accumulated over 16 DIN chunks
                zt_ps = pszp.tile([R, 512], F32, tag="zt")
                for c in range(DIN // 128):
                    nc.tensor.matmul(zt_ps[:], amixT[c][:],
                                     hTs[c][:],
                                     start=(c == 0), stop=(c == DIN // 128 - 1))
                zt = zp.tile([R, 512], F32R, tag="z")
                nc.vector.tensor_copy(zt[:], zt_ps[:])

                # mm2: delta tile [128, DOUT] per s-tile
                for t in range(4):
                    row0 = (sc * 4 + t) * 128
                    dsb = dp.tile([128, DOUT], F32, tag="d")
                    for oc in range(DOUT // 512):
                        osl = slice(oc * 512, (oc + 1) * 512)
                        dps = psdp.tile([128, 512], F32, tag="dps")
                        nc.tensor.matmul(
                            dps[:], zt[:, t * 128:(t + 1) * 128],
                            bmixT[:, osl])
                        nc.scalar.copy(dsb[:, osl], dps[:])
                    nc.sync.dma_start(out_d[row0:row0 + 128, :], dsb[:])

    nc.compile()
    return nc


def _in_maps(h, alpha, A_bank, B_bank):
    a_flat = np.ascontiguousarray(A_bank.reshape(KR, DIN), dtype=np.float32)
    bt_flat = np.ascontiguousarray(
        B_bank.transpose(0, 2, 1).reshape(KR, DOUT), dtype=np.float32)
    eye = np.eye(R, dtype=np.float32)
    maps = []
    for b in range(B):
        mix = np.kron(alpha[b].astype(np.float32).reshape(K, 1), eye)
        maps.append({
            "hb": np.ascontiguousarray(h[b], dtype=np.float32),
            "mix": np.ascontiguousarray(mix),
            "a_flat": a_flat,
            "bt_flat": bt_flat,
        })
    return maps


def _run(inputs, trace=False):
    if "nc" not in _cache:
        _cache["nc"] = _build_nc()
    nc = _cache["nc"]
    maps = _in_maps(inputs["h"], inputs["alpha"], inputs["A_bank"],
                    inputs["B_bank"])
    res = run_bass_kernel_spmd(nc, maps, core_ids=list(range(B)), trace=trace)
    out = np.stack([res.results[b]["delta"] for b in range(B)], axis=0)
    return out.astype(np.float32), res


def kernel(**inputs):
    out, _ = _run(inputs, trace=False)
    return out


# revision 4
# speedup vs baseline: 1.2804x; 1.2804x over previous
"""LoRA basis-bank kernel for 8 TRN2 NeuronCores.

Math (per batch b):
    A_mixed  = sum_k alpha[b,k] * A_bank[k]        # [R, DIN]
    B_mixedT = sum_k alpha[b,k] * B_bank[k].T      # [R, DOUT]
    z        = h[b] @ A_mixed.T                    # [S, R]
    delta[b] = z @ B_mixedT                        # [S, DOUT]

Sharding: data-parallel over batch, 1 batch per core; banks replicated.

Device dataflow per core:
  - h is uploaded as bf16 (halves input HBM traffic); delta is written
    bf16 and upcast on host (halves output traffic)
  - mix banks with a matmul against a [K*R, R] block-diagonal alpha
    placement matrix (built on host -- data placement only), float32r
  - transpose h in 128x128 blocks on the tensor engine (bf16: 1 cyc/row
    stream + fast weight load)
  - mm1: zT[r, s] = A_mixT.T @ hT accumulated over DIN chunks, bf16
  - mm2: delta[s, o] = zT.T @ B_mixedT in float32r (keeps B-side and
    accumulation near-fp32)
"""

import ml_dtypes
import numpy as np

import concourse.bacc as bacc
import concourse.bass as bass
import concourse.mybir as mybir
import concourse.tile as tile
from concourse.bass_utils import run_bass_kernel_spmd
from concourse.masks import make_identity

B, S, K, R, DIN, DOUT = 8, 2048, 16, 16, 2048, 2048
KR = K * R  # 256
F32 = mybir.dt.float32
F32R = mybir.dt.float32r
BF16 = mybir.dt.bfloat16

_cache = {}


def _build_nc():
    nc = bacc.Bacc("TRN2", target_bir_lowering=False)

    h_d = nc.dram_tensor("hb", [S, DIN], BF16, kind="ExternalInput")
    mix_d = nc.dram_tensor("mix", [KR, R], F32R, kind="ExternalInput")
    a_d = nc.dram_tensor("a_flat", [KR, DIN], F32R, kind="ExternalInput")
    bt_d = nc.dram_tensor("bt_flat", [KR, DOUT], F32R, kind="ExternalInput")
    out_d = nc.dram_tensor("delta", [S, DOUT], BF16, kind="ExternalOutput")

    NCH = DIN // 128  # 16 transpose chunks
    with tile.TileContext(nc) as tc:
        with (
            tc.tile_pool(name="const", bufs=1) as constp,
            tc.tile_pool(name="banks", bufs=1) as bankp,
            tc.tile_pool(name="hin", bufs=8) as hp,
            tc.tile_pool(name="hT", bufs=2) as hTp,
            tc.tile_pool(name="zz", bufs=2) as zp,
            tc.tile_pool(name="dout", bufs=3) as dp,
            tc.tile_pool(name="pst", bufs=2, space="PSUM") as pstp,
            tc.tile_pool(name="psz", bufs=2, space="PSUM") as pszp,
            tc.tile_pool(name="psd", bufs=3, space="PSUM") as psdp,
        ):
            ident = constp.tile([128, 128], BF16, tag="ident")
            make_identity(nc, ident[:])

            # ---- chunk-0 h loads first so PE can start transposing ----
            hts0 = []
            for t in range(4):
                ht = hp.tile([128, DIN], BF16, tag="h")
                nc.sync.dma_start(ht[:], h_d[t * 128:(t + 1) * 128, :])
                hts0.append(ht)

            # ---- banks + alpha placement matrix (ACT's HWDGE ring) ----
            m_sb = []
            for half in range(2):
                m_t = constp.tile([128, R], F32R, tag=f"m{half}")
                nc.scalar.dma_start(m_t[:], mix_d[half * 128:(half + 1) * 128, :])
                m_sb.append(m_t)
            a_sb, b_sb = [], []
            for half in range(2):
                a_t = bankp.tile([128, DIN], F32R, tag=f"a{half}")
                nc.scalar.dma_start(a_t[:], a_d[half * 128:(half + 1) * 128, :])
                a_sb.append(a_t)
                b_t = bankp.tile([128, DOUT], F32R, tag=f"b{half}")
                nc.scalar.dma_start(b_t[:], bt_d[half * 128:(half + 1) * 128, :])
                b_sb.append(b_t)

            def transpose_chunk(hts):
                """16 psum tiles, each [128i, 512s] = transposes of 4 h tiles."""
                hTs = []
                for c in range(NCH):
                    pt = pstp.tile([128, 512], BF16, tag="pt")
                    for t in range(4):
                        nc.tensor.matmul(
                            pt[:, t * 128:(t + 1) * 128],
                            hts[t][:, c * 128:(c + 1) * 128],
                            ident[:], is_transpose=True)
                    hT = hTp.tile([128, 512], BF16, tag=f"hT{c}")
                    nc.vector.tensor_copy(hT[:], pt[:])
                    hTs.append(hT)
                return hTs

            hTs = transpose_chunk(hts0)

            # ---- mix banks (f32r): overlapped with chunk-0 transposes ----
            amixed = constp.tile([R, DIN], BF16, tag="amixed")
            bmixT = constp.tile([R, DOUT], F32R, tag="bmixT")
            for dst, src in ((amixed, a_sb), (bmixT, b_sb)):
                for c4 in range(DIN // 512):
                    sl = slice(c4 * 512, (c4 + 1) * 512)
                    pmix = pstp.tile([R, 512], F32, tag="pt")
                    nc.tensor.matmul(pmix[:], m_sb[0][:], src[0][:, sl],
                                     start=True, stop=False)
                    nc.tensor.matmul(pmix[:], m_sb[1][:], src[1][:, sl],
                                     start=False, stop=True)
                    nc.vector.tensor_copy(dst[:, sl], pmix[:])

            # ---- A_mixed.T chunks: [128, R] x 16 (bf16 lhsT for mm1) ----
            amixT = []
            for c in range(NCH):
                pat = pszp.tile([128, R], BF16, tag="zt")
                nc.tensor.matmul(pat[:], amixed[:, c * 128:(c + 1) * 128],
                                 ident[:R, :R], is_transpose=True)
                t_sb = constp.tile([128, R], BF16, tag=f"amixT{c}")
                nc.vector.tensor_copy(t_sb[:], pat[:])
                amixT.append(t_sb)

            # ---- main loop over s-chunks of 512 rows ----
            for sc in range(S // 512):
                # mm1: zT [R, 512] accumulated over 16 DIN chunks (bf16)
                zt_ps = pszp.tile([R, 512], F32, tag="zt")
                for c in range(NCH):
                    nc.tensor.matmul(zt_ps[:], amixT[c][:], hTs[c][:],
                                     start=(c == 0), stop=(c == NCH - 1))
                zt = zp.tile([R, 512], F32R, tag="z")
                nc.vector.tensor_copy(zt[:], zt_ps[:])

                # prefetch + transpose next chunk while mm2 runs
                if sc + 1 < S // 512:
                    hts = []
                    for t in range(4):
                        row0 = ((sc + 1) * 4 + t) * 128
                        ht = hp.tile([128, DIN], BF16, tag="h")
                        nc.sync.dma_start(ht[:], h_d[row0:row0 + 128, :])
                        hts.append(ht)
                    hTs = transpose_chunk(hts)

                # mm2: delta tile [128, DOUT] per s-tile (f32r)
                for t in range(4):
                    row0 = (sc * 4 + t) * 128
                    dsb = dp.tile([128, DOUT], BF16, tag="d")
                    for oc in range(DOUT // 512):
                        osl = slice(oc * 512, (oc + 1) * 512)
                        dps = psdp.tile([128, 512], F32, tag="dps")
                        nc.tensor.matmul(
                            dps[:], zt[:, t * 128:(t + 1) * 128],
                            bmixT[:, osl])
                        nc.scalar.copy(dsb[:, osl], dps[:])
                    nc.sync.dma_start(out_d[row0:row0 + 128, :], dsb[:])

    nc.compile()
    return nc


def _in_maps(h, alpha, A_bank, B_bank):
    a_flat = np.ascontiguousarray(A_bank.reshape(KR, DIN), dtype=np.float32)
    bt_flat = np.ascontiguousarray(
        B_bank.transpose(0, 2, 1).reshape(KR, DOUT), dtype=np.float32)
    eye = np.eye(R, dtype=np.float32)
    maps = []
    for b in range(B):
        mix = np.kron(alpha[b].astype(np.float32).reshape(K, 1), eye)
        maps.append({
            "hb": np.ascontiguousarray(h[b]).astype(ml_dtypes.bfloat16),
            "mix": np.ascontiguousarray(mix),
            "a_flat": a_flat,
            "bt_flat": bt_flat,
        })
    return maps


def _run(inputs, trace=False):
    if "nc" not in _cache:
        _cache["nc"] = _build_nc()
    nc = _cache["nc"]
    maps = _in_maps(inputs["h"], inputs["alpha"], inputs["A_bank"],
                    inputs["B_bank"])
    res = run_bass_kernel_spmd(nc, maps, core_ids=list(range(B)), trace=trace)
    out = np.stack([res.results[b]["delta"] for b in range(B)], axis=0)
    return out.astype(np.float32), res


def kernel(**inputs):
    out, _ = _run(inputs, trace=False)
    return out
